# revision 19
# baseline (speedup 1.0000x reference)
"""Multi-head causal attention + output projection, fused into ONE SPMD
launch on 8 Trainium2 NeuronCores.

Problem (full shapes): x [4, 2048, 1024], wq/wk/wv [16, 1024, 64],
w_proj [1024, 1024], b_proj [1024] -> out [4, 2048, 1024].

Strategy (single SPMD launch, no collectives):

Head-parallel attention — each core owns 2 of the 16 heads (= 128 of the
1024 concat channels). Per core: QKV projections contract over C on the
partition dim using a host-pretransposed xT; scores are computed
transposed (scoresT[s, tq] = kT_slice.T @ qT) so the exp'd weights land
directly in the [s, tq] layout the PE needs as the stationary operand of
wei @ v; causal block skipping on both the scores and the wei@v matmuls.

The attention loop runs j-outer over 512-token query windows with the
two heads interleaved: both heads' K=64 score matmuls are issued
back-to-back with stationary kT slices at base partitions 0 and 64 (PE
row-groups 0-1 vs 2-3) and outputs in different PSUM banks, so they
execute concurrently in the 128x128 systolic array. Scores for both
heads share one 2-bank PSUM tile [h0 | h1], so one scalar-engine exp
covers both. The softmax denominator comes free from a ones-column
appended to V (the wei@v matmul computes [v | 1].T @ wei, row 64 = sum
of weights); reciprocal on vector, partition-broadcast via a K=1 ones
matmul, normalization on vector. Matmul operands are bf16 (host-
rounded), accumulation fp32 in PSUM.

Fused output projection — row-parallel: each core multiplies its 128
attention channels by the matching 128 rows of w_proj, producing a FULL
token-major [BT, 1024] fp32 partial product. The 8 partials are summed
on the host (the cross-core all-reduce of the sharding hint, done
off-device), then biased. This removes the second NEFF launch (and its
~75-84 ms fixed dispatch/relay overhead, which dominates the measured
launch time; the kernel body itself is ~0.33 ms) entirely.

kernel() is self-contained: hardcodes shapes, shards on host, runs the
single SPMD NEFF on cores 0-7, reduces + reassembles on host.
"""

import numpy as np
import ml_dtypes

import concourse.bass as bass
import concourse.mybir as mybir
import concourse.tile as tile
from concourse.bass_utils import run_bass_kernel_spmd

B, T, C, H, D = 4, 2048, 1024, 16, 64
NCORES = 8
HPC = H // NCORES          # heads per core = 2
BT = B * T                 # 8192
CB = C // 128              # 8 contraction blocks
NB = T // 128              # 16 s-blocks per batch
F32 = mybir.dt.float32
BF16 = mybir.dt.bfloat16
EXP = mybir.ActivationFunctionType.Exp
BF = ml_dtypes.bfloat16

_CACHE: dict = {}

# tuning knobs (read at build time)
TUNE = {
    "xt_bufs": 18,
    "wei_bufs": 6,
    "sc_bufs": 2,
    "av_bufs": 2,
    "qkv_ps_bufs": 2,
    "outT_bufs": 2,
    "yo_bufs": 4,
    # A/B'd: per-window proj emission (1) measured ~20us SLOWER than
    # end-of-batch (0) — the tile scheduler already interleaves the
    # projection into scalar-paced attention gaps on its own, and forced
    # placement only adds ps_qkv slot contention.
    "pj_interleave": 0,
    # emit the projection partial in bf16 (halves output DMA traffic;
    # the host reduction upcasts to fp32, costing ~0.05% extra rel err)
    "y_bf16": 0,
}


def split_waits(nc, budget=1):
    """Walrus codegen rejects instructions carrying too many semaphore
    waits; offload excess waits onto preceding same-engine NOPs."""
    k = 0
    for bb in nc.main_func.blocks:
        insts = bb.instructions
        i = 0
        while i < len(insts):
            ins = insts[i]
            si = getattr(ins, "sync_info", None)
            if si is not None and si.on_wait and len(si.on_wait) > budget:
                waits = list(si.on_wait)
                extra, keep = waits[:-budget], waits[-budget:]
                pos = i
                for c in range(0, len(extra), budget):
                    nop = mybir.InstNoOp(
                        name=f"I-waitsplit{k}",
                        engine=ins.engine,
                        ins=[],
                        outs=[],
                        sync_info=mybir.SyncInfo(
                            on_wait=extra[c : c + budget], on_update=[]
                        ),
                        bass_nofuse=True,
                    )
                    k += 1
                    insts.insert(pos, nop)
                    pos += 1
                    i += 1
                ins.sync_info = mybir.SyncInfo(
                    on_wait=keep, on_update=list(si.on_update or [])
                )
            i += 1
    return k


def _emit_proj(nc, ps_qkv, opool, outT, wp_sb, y, t0, j):
    """Projection partial for the 4 token blocks of query window j."""
    ydt = BF16 if TUNE["y_bf16"] else F32
    for tb in range(4 * j, 4 * j + 4):
        for oh in range(2):
            pj = ps_qkv.tile(
                [128, 512], F32, tag="ps_qkv", name=f"pj{tb}_{oh}"
            )
            nc.tensor.matmul(
                pj[:],
                outT[:, 128 * tb : 128 * (tb + 1)],
                wp_sb[:, 512 * oh : 512 * (oh + 1)],
                start=True,
                stop=True,
            )
            yo = opool.tile([128, 512], ydt, tag="yo", bufs=TUNE["yo_bufs"])
            nc.any.tensor_copy(yo[:], pj[:])
            nc.sync.dma_start(
                y[
                    t0 + 128 * tb : t0 + 128 * (tb + 1),
                    512 * oh : 512 * (oh + 1),
                ],
                yo[:],
            )


def _build_fused(split=True, reps=1):
    nc = bass.Bass()

    xT = nc.dram_tensor("xT", [C, BT], BF16, kind="ExternalInput")
    wq2 = nc.dram_tensor("wq2", [C, 128], BF16, kind="ExternalInput")
    wk2 = nc.dram_tensor("wk2", [C, 128], BF16, kind="ExternalInput")
    wv2 = nc.dram_tensor("wv2", [C, 128], BF16, kind="ExternalInput")
    wp = nc.dram_tensor("wp", [128, C], BF16, kind="ExternalInput")
    y = nc.dram_tensor(
        "y", [BT, C], BF16 if TUNE["y_bf16"] else F32, kind="ExternalOutput"
    )

    ident_d = nc.inline_tensor(np.eye(128, dtype=BF), name="ident")
    # mask[s, tq] = 1 where s <= tq (keep); applied to the diagonal block
    mask_d = nc.inline_tensor(
        np.triu(np.ones((128, 128), dtype=BF)), name="mask"
    )
    ones_d = nc.inline_tensor(np.ones((1, 64), dtype=BF), name="ones64")

    with tile.TileContext(nc) as tc:
        with (
            tc.tile_pool(name="wpool", bufs=1) as wpool,
            tc.tile_pool(name="qkv", bufs=2) as qkv_pool,
            tc.tile_pool(name="xp", bufs=6) as xpool,
            tc.tile_pool(name="wei", bufs=TUNE["wei_bufs"]) as wei_pool,
            tc.tile_pool(name="small", bufs=4) as spool,
            tc.tile_pool(name="outp", bufs=TUNE["outT_bufs"]) as opool,
            tc.tile_pool(name="ps_qkv", bufs=TUNE["qkv_ps_bufs"], space="PSUM") as ps_qkv,
            tc.tile_pool(name="ps_sc", bufs=TUNE["sc_bufs"], space="PSUM") as ps_sc,
            tc.tile_pool(name="ps_av", bufs=TUNE["av_bufs"], space="PSUM") as ps_av,
        ):
            wq_sb = wpool.tile([128, CB, 128], BF16)
            wk_sb = wpool.tile([128, CB, 128], BF16)
            wv_sb = wpool.tile([128, CB, 128], BF16)
            nc.sync.dma_start(wq_sb[:], wq2[:].rearrange("(b p) m -> p b m", p=128))
            nc.sync.dma_start(wk_sb[:], wk2[:].rearrange("(b p) m -> p b m", p=128))
            nc.sync.dma_start(wv_sb[:], wv2[:].rearrange("(b p) m -> p b m", p=128))
            wp_sb = wpool.tile([128, C], BF16)
            nc.sync.dma_start(wp_sb[:], wp[:])
            ident = wpool.tile([128, 128], BF16)
            nc.sync.dma_start(ident[:], ident_d[:])
            mask = wpool.tile([128, 128], BF16)
            nc.sync.dma_start(mask[:], mask_d[:])
            ones64 = wpool.tile([1, 64], BF16)
            nc.sync.dma_start(ones64[:], ones_d[:])

            rep_ctx = tc.For_i(0, reps, 1) if reps > 1 else None
            if rep_ctx is not None:
                rep_ctx.__enter__()
            for b in range(B):
                t0 = b * T
                # ---- QKV for batch b ----
                qT = qkv_pool.tile([128, T], BF16, tag="qT")
                kT = qkv_pool.tile([128, T], BF16, tag="kT")
                v2 = qkv_pool.tile([128, NB, 130], BF16, tag="v2")
                # ones columns at 64 (head 0) and 129 (head 1)
                nc.gpsimd.memset(v2[:, :, 64:65], 1.0)
                nc.gpsimd.memset(v2[:, :, 129:130], 1.0)

                for tch in range(T // 512):
                    tc0 = t0 + 512 * tch
                    xts = []
                    for cb in range(CB):
                        xt = xpool.tile([128, 512], BF16, tag="xt", bufs=TUNE["xt_bufs"])
                        nc.sync.dma_start(
                            xt[:], xT[128 * cb : 128 * (cb + 1), tc0 : tc0 + 512]
                        )
                        xts.append(xt)
                    for w_sb, dst in ((wq_sb, qT), (wk_sb, kT)):
                        ps = ps_qkv.tile([128, 512], F32, tag="ps_qkv")
                        for cb in range(CB):
                            nc.tensor.matmul(
                                ps[:],
                                w_sb[:, cb],
                                xts[cb][:],
                                start=(cb == 0),
                                stop=(cb == CB - 1),
                            )
                        nc.any.tensor_copy(
                            dst[:, 512 * tch : 512 * (tch + 1)], ps[:]
                        )
                    # v (both heads packed): vT2 then PE-transpose to [t, d]
                    ps = ps_qkv.tile([128, 512], F32, tag="ps_qkv")
                    for cb in range(CB):
                        nc.tensor.matmul(
                            ps[:],
                            wv_sb[:, cb],
                            xts[cb][:],
                            start=(cb == 0),
                            stop=(cb == CB - 1),
                        )
                    vt = xpool.tile([128, 512], BF16, tag="vt", bufs=3)
                    nc.any.tensor_copy(vt[:], ps[:])
                    pst = ps_qkv.tile([128, 512], BF16, tag="ps_qkv")
                    for tb in range(4):
                        nc.tensor.transpose(
                            pst[:, 128 * tb : 128 * (tb + 1)],
                            vt[:, 128 * tb : 128 * (tb + 1)],
                            ident[:],
                        )
                    pst3 = pst[:].rearrange("p (t d) -> p t d", d=128)
                    sb0 = 4 * tch
                    nc.vector.tensor_copy(
                        v2[:, sb0 : sb0 + 4, 0:64], pst3[:, :, 0:64]
                    )
                    nc.vector.tensor_copy(
                        v2[:, sb0 : sb0 + 4, 65:129], pst3[:, :, 64:128]
                    )

                # ---- attention for batch b ----
                # j-outer over 512-token query windows; the two heads are
                # interleaved so their K=64 score matmuls (stationary kT in
                # PE row-groups 0-1 vs 2-3, outputs in different PSUM banks)
                # run concurrently in the systolic array.
                outT = opool.tile([128, T], BF16, tag="outT")
                for j in range(T // 512):
                    av = {}
                    for h in range(2):
                        av[h] = ps_av.tile(
                            [128, 512], F32, tag="av", name=f"av{h}_{j}"
                        )
                    i_last = 4 * j + 3
                    for i in range(4 * j + 4):
                        a = max(0, 128 * i - 512 * j)
                        # scores for both heads packed [h0 | h1] in one
                        # 2-bank PSUM tile
                        ps = ps_sc.tile([128, 1024], F32, tag="sc")
                        for h in range(2):
                            nc.tensor.matmul(
                                ps[:, 512 * h + a : 512 * h + 512],
                                kT[64 * h : 64 * h + 64, 128 * i : 128 * (i + 1)],
                                qT[64 * h : 64 * h + 64, 512 * j + a : 512 * (j + 1)],
                                start=True,
                                stop=True,
                            )
                        wt = wei_pool.tile([128, 1024], BF16, tag="wei")
                        if a == 0:
                            nc.scalar.activation(wt[:], ps[:], EXP)
                        else:
                            nc.scalar.activation(wt[:, a:512], ps[:, a:512], EXP)
                            nc.scalar.activation(
                                wt[:, 512 + a : 1024], ps[:, 512 + a : 1024], EXP
                            )
                        if i >= 4 * j:
                            # diagonal block: window cols [a, a+128)
                            for h in range(2):
                                nc.vector.tensor_mul(
                                    wt[:, 512 * h + a : 512 * h + a + 128],
                                    wt[:, 512 * h + a : 512 * h + a + 128],
                                    mask[:],
                                )
                        for h in range(2):
                            nc.tensor.matmul(
                                av[h][0:65, a:512],
                                v2[:, i, 65 * h : 65 * h + 65],
                                wt[:, 512 * h + a : 512 * h + 512],
                                start=(i == 0),
                                stop=(i == i_last),
                            )
                    for h in range(2):
                        # reciprocal of the denominator row, then broadcast it
                        # across partitions 64..127 of the same PSUM bank via
                        # a K=1 ones matmul, and normalize (DVE reads at most
                        # one PSUM operand, so copy numerators to SBUF first).
                        r = spool.tile([1, 512], BF16, tag="recip")
                        with nc.allow_low_precision(
                            reason="softmax recip in bf16"
                        ):
                            nc.vector.reciprocal(r[:], av[h][64:65, :])
                        nc.tensor.matmul(
                            av[h][64:128, :],
                            ones64[:],
                            r[:],
                            start=True,
                            stop=True,
                        )
                        ot_sl = outT[64 * h : 64 * h + 64, 512 * j : 512 * (j + 1)]
                        nc.any.tensor_copy(ot_sl, av[h][0:64, :])
                        nc.vector.tensor_mul(ot_sl, ot_sl, av[h][64:128, :])

                    # ---- fused row-parallel projection partial ----
                    # y_part[t, mo] = sum_ch outT[ch, t] * wp[ch, mo]
                    # (token-major output: no transpose needed on host)
                    if TUNE["pj_interleave"]:
                        _emit_proj(nc, ps_qkv, opool, outT, wp_sb, y, t0, j)
                if not TUNE["pj_interleave"]:
                    for j2 in range(T // 512):
                        _emit_proj(nc, ps_qkv, opool, outT, wp_sb, y, t0, j2)

            if rep_ctx is not None:
                rep_ctx.__exit__(None, None, None)

    if split:
        split_waits(nc)
    return nc


def _get_nc_fused():
    if "fused" not in _CACHE:
        _CACHE["fused"] = _build_fused()
    return _CACHE["fused"]


def make_in_maps(x, wq, wk, wv, w_proj):
    xT = np.ascontiguousarray(
        np.asarray(x, np.float32).reshape(BT, C).T
    ).astype(BF)
    scale = np.float32(C) ** -0.5
    wpf = np.asarray(w_proj, np.float32)
    in_maps = []
    for c in range(NCORES):
        h0, h1 = HPC * c, HPC * c + 1
        in_maps.append(
            {
                "xT": xT,
                "wq2": np.ascontiguousarray(
                    np.concatenate([wq[h0] * scale, wq[h1] * scale], axis=1)
                ).astype(BF),
                "wk2": np.ascontiguousarray(
                    np.concatenate([wk[h0], wk[h1]], axis=1)
                ).astype(BF),
                "wv2": np.ascontiguousarray(
                    np.concatenate([wv[h0], wv[h1]], axis=1)
                ).astype(BF),
                "wp": np.ascontiguousarray(
                    wpf[128 * c : 128 * (c + 1), :]
                ).astype(BF),
            }
        )
    return in_maps


def assemble_output(results, b_proj):
    acc = np.asarray(results[0]["y"]).astype(np.float32)
    for c in range(1, NCORES):
        acc += np.asarray(results[c]["y"]).astype(np.float32)
    out = acc.reshape(B, T, C)
    out += np.asarray(b_proj, np.float32)
    return out


def kernel(x, wq, wk, wv, w_proj, b_proj):
    wq = np.asarray(wq, np.float32)
    wk = np.asarray(wk, np.float32)
    wv = np.asarray(wv, np.float32)

    res = run_bass_kernel_spmd(
        _get_nc_fused(),
        make_in_maps(x, wq, wk, wv, w_proj),
        core_ids=list(range(NCORES)),
    )
    return assemble_output(res.results, b_proj)
